# revision 31
# baseline (speedup 1.0000x reference)
"""Trainium2 Bass kernel v2 for nn_MultiHeadAttention_8667244003725.

B=4, S=1024, E=1024, H=16, D=64.  Same sharding as baseline:
8 cores = 4 batches x 2 head-groups; core c -> out[b, :, 512g:512g+512],
b=c//2, g=c%2.  No collectives.

v2 changes vs baseline (173us):
- scores row-tiled: both heads of a pair run CONCURRENTLY on the PE
  (K=64 contractions at tile rows 0-63 / 64-127) into one [128,1024] slab.
- one exp per (pair,qc,kb) covering both heads via strided AP; one DVE
  tri-mask per diagonal pair (broadcast AP).
- QKV jobs for later pairs emitted as filler units inside the attention
  chain so the PE never idles on exp -> HAM stays at 2.4 GHz.
- transposes: f32 [65,128] blocks into a shared psum ring; normalize via
  one reciprocal [128,4] + one broadcast tensor_mul per (head, qc).
- weight DMAs off the scalar queue; V/Q/K psum drains on gpsimd/scalar.
"""

import sys

if '/opt/trn_rl_repo' not in sys.path:
    sys.path.insert(0, '/opt/trn_rl_repo')

import numpy as np

import concourse.bass as bass
import concourse.mybir as mybir
import concourse.tile as tile
from concourse.masks import make_identity

F32 = mybir.dt.float32
BF = mybir.dt.bfloat16
AF = mybir.ActivationFunctionType
MUL = mybir.AluOpType.mult

S = 1024
E = 1024
D = 64
HC = 8            # heads per core
NO = 512          # output columns per core


def _split_sync_waits(nc, limit=1):
    """Walrus here rejects >1 sem-wait per instruction; hoist extras onto
    same-engine no-ops."""
    n = 0
    for f in nc.m.functions:
        for bb in f.blocks:
            out = []
            for ins in bb.instructions:
                si = ins.sync_info
                waits = list(si.on_wait) if si is not None else []
                if len(waits) > limit:
                    excess, keep = waits[:-limit], waits[-limit:]
                    for i in range(0, len(excess), limit):
                        grp = excess[i:i + limit]
                        n += 1
                        out.append(mybir.InstNoOp(
                            name=f'I-synsplit-{n}', ins=[], outs=[],
                            engine=ins.engine,
                            sync_info=mybir.SyncInfo(on_wait=list(grp),
                                                     on_update=[])))
                    si.on_wait = keep
                out.append(ins)
            bb.instructions = out
    return n


def build_nc(split_waits=True, debug=False):
    nc = bass.Bass()
    xb = nc.dram_tensor('xb', [E, S], BF, kind='ExternalInput')   # x[b]^T
    wq = nc.dram_tensor('wq', [E, HC * D], BF, kind='ExternalInput')
    wk = nc.dram_tensor('wk', [E, HC * D], BF, kind='ExternalInput')
    wv = nc.dram_tensor('wv', [E, HC * D], BF, kind='ExternalInput')
    wo = nc.dram_tensor('wo', [E, E], BF, kind='ExternalInput')   # W_O^T
    out = nc.dram_tensor('out', [E, NO], F32, kind='ExternalOutput')
    dbg = None
    if debug:
        dbg = {
            'dC': nc.dram_tensor('dC', [128, 8 * NO], BF,
                                 kind='ExternalOutput'),
            'dQ': nc.dram_tensor('dQ', [4 * 128, S], BF,
                                 kind='ExternalOutput'),
            'dK': nc.dram_tensor('dK', [4 * 128, S], BF,
                                 kind='ExternalOutput'),
            'dV': nc.dram_tensor('dV', [8 * 128, HC * (D + 1)], BF,
                                 kind='ExternalOutput'),
        }

    with tile.TileContext(nc) as tc:
        _emit(nc, tc, xb, wq, wk, wv, wo, out, dbg=dbg)
    if split_waits:
        _split_sync_waits(nc)
    return nc


def _copier(eng):
    """Uniform copy callable: scalar uses activation-Copy, DVE tensor_copy.
    (GPSIMD cannot read PSUM on trn2.)"""
    if hasattr(eng, 'tensor_copy'):
        return eng.tensor_copy
    return eng.copy


def _emit(nc, tc, xb, wq, wk, wv, wo, out, dbg=None):
    with (
        tc.tile_pool(name='const', bufs=1) as constp,
        tc.tile_pool(name='big', bufs=1) as bigp,      # xT, weights, WOT, C
        tc.tile_pool(name='qk', bufs=1) as qkp,
        tc.tile_pool(name='vall', bufs=1) as vallp,
        tc.tile_pool(name='sc', bufs=2, space='PSUM') as scp,   # 4 banks
        tc.tile_pool(name='av', bufs=3, space='PSUM') as avp,   # 3 banks
        tc.tile_pool(name='fl', bufs=1, space='PSUM') as flp,   # 1 bank
    ):
        # ---- constants --------------------------------------------------
        identf = constp.tile([128, 128], F32, tag='identf')
        make_identity(nc, identf[:])
        identb = constp.tile([128, 128], BF, tag='identb')
        make_identity(nc, identb[:])
        ones8 = constp.tile([128, 8], BF, tag='ones8')
        nc.gpsimd.memset(ones8[:], 1.0)
        # tri[k, q] = 1 where q >= k else 0 (multiplicative causal mask)
        tri = constp.tile([128, 128], BF, tag='tri')
        nc.gpsimd.memset(tri[:], 1.0)
        nc.gpsimd.affine_select(
            out=tri[:], in_=tri[:], compare_op=mybir.AluOpType.is_ge,
            fill=0.0, base=0, channel_multiplier=-1, pattern=[[1, 128]])
        # ---- input DMAs -------------------------------------------------
        # Per-chunk tiles so consumers depend on exactly the chunk they
        # read (whole-tile deps made the first matmul wait ~16us).
        # Queues:  sync x4..7 + wk0..7;  scalar wq4..7,0..3;
        #          gpsimd x0..3 + wv + wo.
        xTc = [bigp.tile([128, S], BF, tag=f'x{ec}', name=f'xT{ec}')
               for ec in range(8)]
        wqc = [bigp.tile([128, 512], BF, tag=f'wq{ec}', name=f'wq{ec}')
               for ec in range(8)]
        wkc = [bigp.tile([128, 512], BF, tag=f'wk{ec}', name=f'wk{ec}')
               for ec in range(8)]
        wvc = [bigp.tile([128, 512], BF, tag=f'wv{ec}', name=f'wv{ec}')
               for ec in range(8)]
        for ec in (4, 5, 6, 7, 0, 1, 2, 3):
            nc.sync.dma_start(xTc[ec][:], xb[ec * 128:(ec + 1) * 128, :])
            nc.scalar.dma_start(wqc[ec][:], wq[ec * 128:(ec + 1) * 128, :])
            nc.gpsimd.dma_start(wvc[ec][:], wv[ec * 128:(ec + 1) * 128, :])
        for ec in range(8):
            nc.sync.dma_start(wkc[ec][:], wk[ec * 128:(ec + 1) * 128, :])
        # warm the ACT exp table after the scalar-queue DMAs are issued
        warm = constp.tile([1, 2], F32, tag='warm')
        nc.scalar.activation(warm[:], ones8[0:1, 0:2], AF.Exp, scale=0.125)
        xT = [t[:] for t in xTc]
        wqt = [t[:] for t in wqc]
        wkt = [t[:] for t in wkc]
        wvt = [t[:] for t in wvc]

        # ---- persistent SBUF --------------------------------------------
        QT2 = [qkp.tile([128, S], BF, tag=f'q{p}', name=f'QT2_{p}')
               for p in range(4)]
        KT2 = [qkp.tile([128, S], BF, tag=f'k{p}', name=f'KT2_{p}')
               for p in range(4)]
        Vall = [vallp.tile([128, HC * (D + 1)], BF, tag=f'v{st}',
                           name=f'Vall{st}') for st in range(8)]
        CA = bigp.tile([128, 8 * 384], BF, tag='CA', name='CA')  # heads 0-5
        CB = bigp.tile([128, 8 * 128], BF, tag='CB', name='CB')  # heads 6-7
        WOTall = bigp.tile([128, 8 * E], BF, tag='WOT', name='WOTall')
        PB = bigp.tile([128, 8 * 128], F32, tag='PB', name='PBpart')
        # persistent cycled slots (WAR deps give the same ordering a pool
        # ring would, without per-allocation release-semaphore traffic)
        pexp_slots = [bigp.tile([128, 1024], BF, tag=f'pe{i}',
                                name=f'pexp{i}') for i in range(4)]
        otsb_slots = [bigp.tile([128, 512], F32, tag=f'ob{i}',
                                name=f'otsb{i}') for i in range(4)]
        rl_slots = [bigp.tile([128, 4], F32, tag=f'rl{i}', name=f'rl{i}')
                    for i in range(2)]
        ys_slots = [bigp.tile([128, 384], F32, tag=f'ys{i}', name=f'ys{i}')
                    for i in range(4)]
        slot_ctr = {'pe': 0, 'ob': 0, 'rl': 0, 'ys': 0}

        def next_slot(kind, slots):
            i = slot_ctr[kind] % len(slots)
            slot_ctr[kind] += 1
            return slots[i]
        WOT = [WOTall[:, i * E:(i + 1) * E] for i in range(8)]
        for i in range(8):
            nc.gpsimd.dma_start(WOTall[:, i * E:(i + 1) * E],
                                wo[i * 128:(i + 1) * 128, :])

        # ---- job generators (each yield = ~0.4-0.9us of PE work) --------
        def gen_qk(p, wt, dst, copy_eng, ec_order=None, filler=False):
            ecs = ec_order or list(range(8))
            if not filler:
                ps = scp.tile([128, 1024], F32, tag='sc',
                              name=f'qk_{p}_{wt is wkt}')
                for i, ec in enumerate(ecs):
                    for s2 in range(2):
                        nc.tensor.matmul(
                            ps[:, s2 * 512:(s2 + 1) * 512],
                            wt[ec][:, p * 128:(p + 1) * 128],
                            xT[ec][:, s2 * 512:(s2 + 1) * 512],
                            start=(i == 0), stop=(i == 7))
                    if i % 2 == 1 and i < 7:
                        yield
                copy_eng(dst[:], ps[:])
                yield
                return
            # filler mode: isolated single-bank pool, two half chains
            for s2 in range(2):
                ps = flp.tile([128, 512], F32, tag='fl',
                              name=f'qkf_{p}_{wt is wkt}_{s2}')
                for i, ec in enumerate(ecs):
                    nc.tensor.matmul(
                        ps[:], wt[ec][:, p * 128:(p + 1) * 128],
                        xT[ec][:, s2 * 512:(s2 + 1) * 512],
                        start=(i == 0), stop=(i == 7))
                    if i % 2 == 1 and i < 7:
                        yield
                copy_eng(dst[:, s2 * 512:(s2 + 1) * 512], ps[:])
                yield

        def gen_v(st):
            ps = scp.tile([128, 1024], F32, tag='sc', name=f'v_{st}')
            for i in range(8):
                nc.tensor.matmul(ps[:, 0:512],
                                 xT[i][:, st * 128:(st + 1) * 128],
                                 wvt[i], start=(i == 0), stop=(i == 7))
                if i % 2 == 1 and i < 7:
                    yield
            v3 = Vall[st][:].rearrange('p (h d) -> p h d', h=HC)
            cp = _copier(nc.vector)
            cp(v3[:, :, 0:D],
               ps[:, 0:512].rearrange('p (h d) -> p h d', h=HC))
            cp(v3[:, :, D:D + 1],
               ones8[:].rearrange('p (h o) -> p h o', o=1))
            yield

        def drain(g):
            for _ in g:
                pass

        from collections import deque
        fillers = deque()

        def fill(n=1):
            k = 0
            while k < n and fillers:
                try:
                    next(fillers[0])
                    k += 1
                except StopIteration:
                    fillers.popleft()

        def gen_projB_p1():
            # proj for heads 6-7, contraction s-blocks 0..3 only (ready
            # after pair-3 qc0); partial parked in SBUF
            for it in range(8):
                ps = flp.tile([128, 512], F32, tag='fl', name=f'pb1_{it}')
                for i in range(4):
                    nc.tensor.matmul(ps[:, 0:128],
                                     WOT[i][:, it * 128:(it + 1) * 128],
                                     CB[:, i * 128:(i + 1) * 128],
                                     start=(i == 0), stop=(i == 3))
                nc.vector.tensor_copy(PB[:, it * 128:(it + 1) * 128],
                                      ps[:, 0:128])
                yield

        def gen_projB_p2():
            for it in range(8):
                ps = scp.tile([128, 1024], F32, tag='sc', name=f'pb2_{it}')
                for i in range(4, 8):
                    nc.tensor.matmul(ps[:, 0:128],
                                     WOT[i][:, it * 128:(it + 1) * 128],
                                     CB[:, i * 128:(i + 1) * 128],
                                     start=(i == 4), stop=(i == 7))
                ys = next_slot('ys', ys_slots)
                nc.vector.tensor_add(ys[:, 0:128], ps[:, 0:128],
                                     PB[:, it * 128:(it + 1) * 128])
                dq = nc.sync if it % 2 == 0 else nc.scalar
                dq.dma_start(out[it * 128:(it + 1) * 128, 384:512],
                             ys[:, 0:128])
                yield

        def gen_proj_half(half):
            """Output projection: half 0 = 384 cols (heads 0-5, complete
            after pair 2 -> filler for pair 3); half 1 = 128 cols (heads
            6-7) at the end.  One whole it-chunk per unit."""
            Csrc, j0, w = (CA, 0, 384) if half == 0 else (CB, 384, 128)
            for it in range(8):
                if half == 0:
                    ps = flp.tile([128, 512], F32, tag='fl',
                                  name=f'proj{half}_{it}')
                else:
                    ps = scp.tile([128, 1024], F32, tag='sc',
                                  name=f'proj{half}_{it}')
                for i in range(8):
                    nc.tensor.matmul(ps[:, 0:w],
                                     WOT[i][:, it * 128:(it + 1) * 128],
                                     Csrc[:, i * w:(i + 1) * w],
                                     start=(i == 0), stop=(i == 7))
                ys = next_slot('ys', ys_slots)
                cp = _copier(nc.vector if (half == 0 or it % 2 == 0)
                             else nc.scalar)
                cp(ys[:, 0:w], ps[:, 0:w])
                dq = nc.sync if it % 2 == 0 else nc.scalar
                dq.dma_start(out[it * 128:(it + 1) * 128, j0:j0 + w],
                             ys[:, 0:w])
                yield

        # ---- prologue: Q0 V0..7 K0 (PE-dense, warms HAM) ----------------
        # Rule (empirical, HW): a filler's output must be fully emitted
        # before the first instruction of the window that consumes it.
        # So V is all in the prologue; QK(p+1) fills pair p's windows;
        # proj half A (heads 0-3, complete after pair 1) fills pair 3.
        # HAM warm-up: keep the PE busy while the first input chunks are
        # still in flight so the clock gate opens before the real work.
        wps = scp.tile([128, 1024], F32, tag='sc', name='warmps')
        for i in range(12):
            nc.tensor.matmul(wps[:, 0:128], identb[:], identb[:],
                             start=True, stop=True)
        g0 = gen_qk(0, wqt, QT2[0], _copier(nc.scalar),
                    ec_order=[4, 5, 6, 7, 0, 1, 2, 3])
        for _ in g0:
            for i in range(6):
                nc.tensor.matmul(wps[:, 0:128], identb[:], identb[:],
                                 start=True, stop=True)
        for st in range(4):
            drain(gen_v(st))
        drain(gen_qk(0, wkt, KT2[0], _copier(nc.scalar)))

        # ---- fillers for the attention phase ----------------------------
        per_pair_fillers = {
            0: [gen_v(4), gen_v(5), gen_v(6), gen_v(7),
                gen_qk(1, wqt, QT2[1], _copier(nc.scalar), filler=True),
                gen_qk(1, wkt, KT2[1], _copier(nc.scalar), filler=True)],
            1: [gen_qk(2, wqt, QT2[2], _copier(nc.vector), filler=True),
                gen_qk(2, wkt, KT2[2], _copier(nc.vector), filler=True)],
            2: [gen_qk(3, wqt, QT2[3], _copier(nc.vector), filler=True),
                gen_qk(3, wkt, KT2[3], _copier(nc.vector), filler=True)],
            3: [gen_proj_half(0), gen_projB_p1()],
        }

        # ---- attention ---------------------------------------------------
        CA3 = CA[:].rearrange('p (st c) -> p st c', st=8)
        CB3 = CB[:].rearrange('p (st c) -> p st c', st=8)
        tri_bc = tri[:, None, :].broadcast_to((128, 2, 128))

        def emit_scores(p, qc, kb):
            """Row-tiled pair of score matmuls + exp + causal mask.
            Returns the pexp tile."""
            j = kb - 4 * qc
            off = 128 * j if j >= 0 else 0
            stp = scp.tile([128, 1024], F32, tag='sc',
                           name=f'stp_{p}_{qc}_{kb}')
            for hh in range(2):
                nc.tensor.matmul(
                    stp[:, hh * 512 + off:(hh + 1) * 512],
                    KT2[p][64 * hh:64 * hh + 64, kb * 128:(kb + 1) * 128],
                    QT2[p][64 * hh:64 * hh + 64,
                           qc * 512 + off:(qc + 1) * 512],
                    start=True, stop=True)
            pexp = next_slot('pe', pexp_slots)
            stp3 = stp[:].rearrange('p (two c) -> p two c', two=2)
            pexp3 = pexp[:].rearrange('p (two c) -> p two c', two=2)
            nc.scalar.activation(pexp3[:, :, off:512], stp3[:, :, off:512],
                                 AF.Exp, scale=0.125)
            if j >= 0:
                nc.vector.tensor_mul(pexp3[:, :, off:off + 128],
                                     pexp3[:, :, off:off + 128], tri_bc)
            return pexp, off

        pending_tp = []

        def emit_pending_tp():
            for fn in pending_tp:
                fn()
            pending_tp.clear()

        def defer_transposes(p, qc, otsbs):
            def go():
                for hh in range(2):
                    h = 2 * p + hh
                    tp = avp.tile([128, 512], F32, tag='av',
                                  name=f'tp_{p}_{qc}_{hh}')
                    for qb in range(4):
                        nc.tensor.transpose(
                            tp[:, qb * 65:qb * 65 + 65],
                            otsbs[hh][:D + 1, qb * 128:(qb + 1) * 128],
                            identf[:D + 1, :D + 1])
                    tp3 = tp[:, 0:260].rearrange('p (qb c) -> p qb c', qb=4)
                    rl = next_slot('rl', rl_slots)
                    nc.vector.reciprocal(rl[:], tp3[:, :, D])
                    Cd3, hl = (CA3, h) if h < 6 else (CB3, h - 6)
                    nc.vector.tensor_mul(
                        Cd3[:, qc * 4:(qc + 1) * 4, hl * D:(hl + 1) * D],
                        tp3[:, :, 0:D],
                        rl[:, :, None].broadcast_to((128, 4, D)))
            pending_tp.append(go)

        for p in range(4):
            if p >= 1:
                while fillers:   # QK(p) leftovers must land before pair p
                    fill(1)
            for g in per_pair_fillers.get(p, []):
                fillers.append(g)
            for qc in range(2):
                if p == 0 and qc == 1:
                    # V4..7 must be fully emitted before qc1 reads them
                    while len(fillers) > 2:
                        fill(1)
                kbs = list(range(4 * qc + 4))
                n = len(kbs)
                pend = {}
                pend[0] = emit_scores(p, qc, 0)
                if n > 1:
                    pend[1] = emit_scores(p, qc, 1)
                # previous window's transposes land here, covering the
                # exp latency of this window's first blocks
                emit_pending_tp()
                ots = [avp.tile([128, 512], F32, tag='av',
                                name=f'ot_{p}_{qc}_{hh}') for hh in range(2)]
                for t in range(n):
                    fill(1)
                    pexp, off = pend.pop(t)
                    for hh in range(2):
                        nc.tensor.matmul(
                            ots[hh][:D + 1, off:512],
                            Vall[t][:, (2 * p + hh) * (D + 1):
                                    (2 * p + hh + 1) * (D + 1)],
                            pexp[:, hh * 512 + off:(hh + 1) * 512],
                            start=(t == 0), stop=(t == n - 1))
                    if t + 2 < n:
                        pend[t + 2] = emit_scores(p, qc, t + 2)
                # drain accumulators now (releases av tiles); defer the
                # PE transposes into the next window
                otsbs = []
                for hh in range(2):
                    otsb = next_slot('ob', otsb_slots)
                    nc.vector.tensor_copy(otsb[:D + 1, :],
                                          ots[hh][:D + 1, :])
                    otsbs.append(otsb)
                defer_transposes(p, qc, otsbs)

        while fillers:       # leftover phase-1 units (PE-dense)
            fill(1)
        emit_pending_tp()
        drain(gen_projB_p2())

        if dbg is not None:
            nc.sync.dma_start(dbg['dC'][:, 0:8 * 384], CA[:])
            nc.sync.dma_start(dbg['dC'][:, 8 * 384:], CB[:])
            for p4 in range(4):
                nc.sync.dma_start(dbg['dQ'][p4 * 128:(p4 + 1) * 128, :],
                                  QT2[p4][:])
                nc.sync.dma_start(dbg['dK'][p4 * 128:(p4 + 1) * 128, :],
                                  KT2[p4][:])
            for st in range(8):
                nc.sync.dma_start(dbg['dV'][st * 128:(st + 1) * 128, :],
                                  Vall[st][:])




_NC_CACHE = None


def _get_nc():
    global _NC_CACHE
    if _NC_CACHE is None:
        _NC_CACHE = build_nc()
    return _NC_CACHE


def make_in_maps(x, Wq, Wk, Wv, W_O):
    import ml_dtypes
    bf = ml_dtypes.bfloat16
    x = np.asarray(x, np.float32)
    xT_by_b = [np.ascontiguousarray(x[b].T.astype(bf)) for b in range(4)]
    W_O = np.ascontiguousarray(np.asarray(W_O, np.float32).T.astype(bf))
    in_maps = []
    for c in range(8):
        b, g = c // 2, c % 2
        hsl = slice(HC * g, HC * g + HC)
        in_maps.append({
            'xb': xT_by_b[b],
            'wq': np.ascontiguousarray(
                np.asarray(Wq, np.float32)[hsl].transpose(1, 0, 2)
                .reshape(E, HC * D).astype(bf)),
            'wk': np.ascontiguousarray(
                np.asarray(Wk, np.float32)[hsl].transpose(1, 0, 2)
                .reshape(E, HC * D).astype(bf)),
            'wv': np.ascontiguousarray(
                np.asarray(Wv, np.float32)[hsl].transpose(1, 0, 2)
                .reshape(E, HC * D).astype(bf)),
            'wo': W_O,
        })
    return in_maps


def kernel(x, Wq, Wk, Wv, W_O):
    from concourse.bass_utils import run_bass_kernel_spmd
    nc = _get_nc()
    in_maps = make_in_maps(x, Wq, Wk, Wv, W_O)
    res = run_bass_kernel_spmd(nc, in_maps, list(range(8)))
    full = np.empty((4, E, E), np.float32)
    for c in range(8):
        b, g = c // 2, c % 2
        full[b, :, NO * g:NO * g + NO] = res.results[c]['out']
    return full


# revision 33
# speedup vs baseline: 1.0363x; 1.0363x over previous
"""Trainium2 Bass kernel v2 for nn_MultiHeadAttention_8667244003725.

B=4, S=1024, E=1024, H=16, D=64.  Same sharding as baseline:
8 cores = 4 batches x 2 head-groups; core c -> out[b, :, 512g:512g+512],
b=c//2, g=c%2.  No collectives.

v2 changes vs baseline (173us):
- scores row-tiled: both heads of a pair run CONCURRENTLY on the PE
  (K=64 contractions at tile rows 0-63 / 64-127) into one [128,1024] slab.
- one exp per (pair,qc,kb) covering both heads via strided AP; one DVE
  tri-mask per diagonal pair (broadcast AP).
- QKV jobs for later pairs emitted as filler units inside the attention
  chain so the PE never idles on exp -> HAM stays at 2.4 GHz.
- transposes: f32 [65,128] blocks into a shared psum ring; normalize via
  one reciprocal [128,4] + one broadcast tensor_mul per (head, qc).
- weight DMAs off the scalar queue; V/Q/K psum drains on gpsimd/scalar.
"""

import sys

if '/opt/trn_rl_repo' not in sys.path:
    sys.path.insert(0, '/opt/trn_rl_repo')

import numpy as np

import concourse.bass as bass
import concourse.mybir as mybir
import concourse.tile as tile
from concourse.masks import make_identity

F32 = mybir.dt.float32
BF = mybir.dt.bfloat16
AF = mybir.ActivationFunctionType
MUL = mybir.AluOpType.mult

S = 1024
E = 1024
D = 64
HC = 8            # heads per core
NO = 512          # output columns per core


def _split_sync_waits(nc, limit=1):
    """Walrus here rejects >1 sem-wait per instruction; hoist extras onto
    same-engine no-ops."""
    n = 0
    for f in nc.m.functions:
        for bb in f.blocks:
            out = []
            for ins in bb.instructions:
                si = ins.sync_info
                waits = list(si.on_wait) if si is not None else []
                if len(waits) > limit:
                    excess, keep = waits[:-limit], waits[-limit:]
                    for i in range(0, len(excess), limit):
                        grp = excess[i:i + limit]
                        n += 1
                        out.append(mybir.InstNoOp(
                            name=f'I-synsplit-{n}', ins=[], outs=[],
                            engine=ins.engine,
                            sync_info=mybir.SyncInfo(on_wait=list(grp),
                                                     on_update=[])))
                    si.on_wait = keep
                out.append(ins)
            bb.instructions = out
    return n


def build_nc(split_waits=True, debug=False):
    nc = bass.Bass()
    xb = nc.dram_tensor('xb', [E, S], BF, kind='ExternalInput')   # x[b]^T
    wq = nc.dram_tensor('wq', [E, HC * D], BF, kind='ExternalInput')
    wk = nc.dram_tensor('wk', [E, HC * D], BF, kind='ExternalInput')
    wv = nc.dram_tensor('wv', [E, HC * D], BF, kind='ExternalInput')
    wo = nc.dram_tensor('wo', [E, E], BF, kind='ExternalInput')   # W_O^T
    out = nc.dram_tensor('out', [E, NO], F32, kind='ExternalOutput')
    dbg = None
    if debug:
        dbg = {
            'dC': nc.dram_tensor('dC', [128, 8 * NO], BF,
                                 kind='ExternalOutput'),
            'dQ': nc.dram_tensor('dQ', [4 * 128, S], BF,
                                 kind='ExternalOutput'),
            'dK': nc.dram_tensor('dK', [4 * 128, S], BF,
                                 kind='ExternalOutput'),
            'dV': nc.dram_tensor('dV', [8 * 128, HC * (D + 1)], BF,
                                 kind='ExternalOutput'),
        }

    with tile.TileContext(nc) as tc:
        _emit(nc, tc, xb, wq, wk, wv, wo, out, dbg=dbg)
    if split_waits:
        _split_sync_waits(nc)
    return nc


def _copier(eng):
    """Uniform copy callable: scalar uses activation-Copy, DVE tensor_copy.
    (GPSIMD cannot read PSUM on trn2.)"""
    if hasattr(eng, 'tensor_copy'):
        return eng.tensor_copy
    return eng.copy


def _emit(nc, tc, xb, wq, wk, wv, wo, out, dbg=None):
    with (
        tc.tile_pool(name='const', bufs=1) as constp,
        tc.tile_pool(name='big', bufs=1) as bigp,      # xT, weights, WOT, C
        tc.tile_pool(name='qk', bufs=1) as qkp,
        tc.tile_pool(name='vall', bufs=1) as vallp,
        tc.tile_pool(name='sc', bufs=2, space='PSUM') as scp,   # 4 banks
        tc.tile_pool(name='av', bufs=3, space='PSUM') as avp,   # 3 banks
        tc.tile_pool(name='fl', bufs=1, space='PSUM') as flp,   # 1 bank
    ):
        # ---- constants --------------------------------------------------
        identf = constp.tile([128, 128], F32, tag='identf')
        make_identity(nc, identf[:])
        identb = constp.tile([128, 128], BF, tag='identb')
        make_identity(nc, identb[:])
        ones8 = constp.tile([128, 8], BF, tag='ones8')
        nc.gpsimd.memset(ones8[:], 1.0)
        # tri[k, q] = 1 where q >= k else 0 (multiplicative causal mask)
        tri = constp.tile([128, 128], BF, tag='tri')
        nc.gpsimd.memset(tri[:], 1.0)
        nc.gpsimd.affine_select(
            out=tri[:], in_=tri[:], compare_op=mybir.AluOpType.is_ge,
            fill=0.0, base=0, channel_multiplier=-1, pattern=[[1, 128]])
        # ---- input DMAs -------------------------------------------------
        # Per-chunk tiles so consumers depend on exactly the chunk they
        # read (whole-tile deps made the first matmul wait ~16us).
        # Queues:  sync x4..7 + wk0..7;  scalar wq4..7,0..3;
        #          gpsimd x0..3 + wv + wo.
        xTc = [bigp.tile([128, S], BF, tag=f'x{ec}', name=f'xT{ec}')
               for ec in range(8)]
        wqc = [bigp.tile([128, 512], BF, tag=f'wq{ec}', name=f'wq{ec}')
               for ec in range(8)]
        wkc = [bigp.tile([128, 512], BF, tag=f'wk{ec}', name=f'wk{ec}')
               for ec in range(8)]
        wvc = [bigp.tile([128, 512], BF, tag=f'wv{ec}', name=f'wv{ec}')
               for ec in range(8)]
        for ec in (4, 5, 6, 7, 0, 1, 2, 3):
            nc.sync.dma_start(xTc[ec][:], xb[ec * 128:(ec + 1) * 128, :])
            nc.scalar.dma_start(wqc[ec][:], wq[ec * 128:(ec + 1) * 128, :])
            nc.gpsimd.dma_start(wvc[ec][:], wv[ec * 128:(ec + 1) * 128, :])
        for ec in range(8):
            nc.sync.dma_start(wkc[ec][:], wk[ec * 128:(ec + 1) * 128, :])
        # warm the ACT exp table after the scalar-queue DMAs are issued
        warm = constp.tile([1, 2], F32, tag='warm')
        nc.scalar.activation(warm[:], ones8[0:1, 0:2], AF.Exp, scale=0.125)
        xT = [t[:] for t in xTc]
        wqt = [t[:] for t in wqc]
        wkt = [t[:] for t in wkc]
        wvt = [t[:] for t in wvc]

        # ---- persistent SBUF --------------------------------------------
        QT2 = [qkp.tile([128, S], BF, tag=f'q{p}', name=f'QT2_{p}')
               for p in range(4)]
        KT2 = [qkp.tile([128, S], BF, tag=f'k{p}', name=f'KT2_{p}')
               for p in range(4)]
        Vall = [vallp.tile([128, HC * (D + 1)], BF, tag=f'v{st}',
                           name=f'Vall{st}') for st in range(8)]
        CA = bigp.tile([128, 8 * 384], BF, tag='CA', name='CA')  # heads 0-5
        CB = bigp.tile([128, 8 * 128], BF, tag='CB', name='CB')  # heads 6-7
        WOTall = bigp.tile([128, 8 * E], BF, tag='WOT', name='WOTall')
        PB = bigp.tile([128, 8 * 128], F32, tag='PB', name='PBpart')
        # persistent cycled slots (WAR deps give the same ordering a pool
        # ring would, without per-allocation release-semaphore traffic)
        pexp_slots = [bigp.tile([128, 1024], BF, tag=f'pe{i}',
                                name=f'pexp{i}') for i in range(4)]
        otsb_slots = [bigp.tile([128, 512], F32, tag=f'ob{i}',
                                name=f'otsb{i}') for i in range(4)]
        rl_slots = [bigp.tile([128, 4], F32, tag=f'rl{i}', name=f'rl{i}')
                    for i in range(2)]
        ys_slots = [bigp.tile([128, 384], F32, tag=f'ys{i}', name=f'ys{i}')
                    for i in range(4)]
        slot_ctr = {'pe': 0, 'ob': 0, 'rl': 0, 'ys': 0}

        def next_slot(kind, slots):
            i = slot_ctr[kind] % len(slots)
            slot_ctr[kind] += 1
            return slots[i]
        WOT = [WOTall[:, i * E:(i + 1) * E] for i in range(8)]
        for i in range(8):
            nc.gpsimd.dma_start(WOTall[:, i * E:(i + 1) * E],
                                wo[i * 128:(i + 1) * 128, :])

        # ---- job generators (each yield = ~0.4-0.9us of PE work) --------
        def gen_qk(p, wt, dst, copy_eng, ec_order=None, filler=False):
            ecs = ec_order or list(range(8))
            if not filler:
                ps = scp.tile([128, 1024], F32, tag='sc',
                              name=f'qk_{p}_{wt is wkt}')
                for i, ec in enumerate(ecs):
                    for s2 in range(2):
                        nc.tensor.matmul(
                            ps[:, s2 * 512:(s2 + 1) * 512],
                            wt[ec][:, p * 128:(p + 1) * 128],
                            xT[ec][:, s2 * 512:(s2 + 1) * 512],
                            start=(i == 0), stop=(i == 7))
                    if i % 2 == 1 and i < 7:
                        yield
                copy_eng(dst[:], ps[:])
                yield
                return
            # filler mode: isolated single-bank pool, two half chains
            for s2 in range(2):
                ps = flp.tile([128, 512], F32, tag='fl',
                              name=f'qkf_{p}_{wt is wkt}_{s2}')
                for i, ec in enumerate(ecs):
                    nc.tensor.matmul(
                        ps[:], wt[ec][:, p * 128:(p + 1) * 128],
                        xT[ec][:, s2 * 512:(s2 + 1) * 512],
                        start=(i == 0), stop=(i == 7))
                    if i % 2 == 1 and i < 7:
                        yield
                copy_eng(dst[:, s2 * 512:(s2 + 1) * 512], ps[:])
                yield

        def gen_v(st):
            ps = scp.tile([128, 1024], F32, tag='sc', name=f'v_{st}')
            for i in range(8):
                nc.tensor.matmul(ps[:, 0:512],
                                 xT[i][:, st * 128:(st + 1) * 128],
                                 wvt[i], start=(i == 0), stop=(i == 7))
                if i % 2 == 1 and i < 7:
                    yield
            v3 = Vall[st][:].rearrange('p (h d) -> p h d', h=HC)
            cp = _copier(nc.vector)
            cp(v3[:, :, 0:D],
               ps[:, 0:512].rearrange('p (h d) -> p h d', h=HC))
            cp(v3[:, :, D:D + 1],
               ones8[:].rearrange('p (h o) -> p h o', o=1))
            yield

        def drain(g):
            for _ in g:
                pass

        from collections import deque
        fillers = deque()

        def fill(n=1):
            k = 0
            while k < n and fillers:
                try:
                    next(fillers[0])
                    k += 1
                except StopIteration:
                    fillers.popleft()

        def gen_projB_p1():
            # proj for heads 6-7, contraction s-blocks 0..3 only (ready
            # after pair-3 qc0); partial parked in SBUF
            for it in range(8):
                ps = flp.tile([128, 512], F32, tag='fl', name=f'pb1_{it}')
                for i in range(4):
                    nc.tensor.matmul(ps[:, 0:128],
                                     WOT[i][:, it * 128:(it + 1) * 128],
                                     CB[:, i * 128:(i + 1) * 128],
                                     start=(i == 0), stop=(i == 3))
                nc.vector.tensor_copy(PB[:, it * 128:(it + 1) * 128],
                                      ps[:, 0:128])
                yield

        def gen_projB_p2():
            for it in range(8):
                ps = scp.tile([128, 1024], F32, tag='sc', name=f'pb2_{it}')
                for i in range(4, 8):
                    nc.tensor.matmul(ps[:, 0:128],
                                     WOT[i][:, it * 128:(it + 1) * 128],
                                     CB[:, i * 128:(i + 1) * 128],
                                     start=(i == 4), stop=(i == 7))
                ys = next_slot('ys', ys_slots)
                nc.vector.tensor_add(ys[:, 0:128], ps[:, 0:128],
                                     PB[:, it * 128:(it + 1) * 128])
                dq = nc.sync if it % 2 == 0 else nc.scalar
                dq.dma_start(out[it * 128:(it + 1) * 128, 384:512],
                             ys[:, 0:128])
                yield

        def gen_proj_half(half):
            """Output projection: half 0 = 384 cols (heads 0-5, complete
            after pair 2 -> filler for pair 3); half 1 = 128 cols (heads
            6-7) at the end.  One whole it-chunk per unit."""
            Csrc, j0, w = (CA, 0, 384) if half == 0 else (CB, 384, 128)
            for it in range(8):
                if half == 0:
                    ps = flp.tile([128, 512], F32, tag='fl',
                                  name=f'proj{half}_{it}')
                else:
                    ps = scp.tile([128, 1024], F32, tag='sc',
                                  name=f'proj{half}_{it}')
                for i in range(8):
                    nc.tensor.matmul(ps[:, 0:w],
                                     WOT[i][:, it * 128:(it + 1) * 128],
                                     Csrc[:, i * w:(i + 1) * w],
                                     start=(i == 0), stop=(i == 7))
                ys = next_slot('ys', ys_slots)
                cp = _copier(nc.vector if (half == 0 or it % 2 == 0)
                             else nc.scalar)
                cp(ys[:, 0:w], ps[:, 0:w])
                dq = nc.sync if it % 2 == 0 else nc.scalar
                dq.dma_start(out[it * 128:(it + 1) * 128, j0:j0 + w],
                             ys[:, 0:w])
                yield

        # ---- prologue: Q0 V0..7 K0 (PE-dense, warms HAM) ----------------
        # Rule (empirical, HW): a filler's output must be fully emitted
        # before the first instruction of the window that consumes it.
        # So V is all in the prologue; QK(p+1) fills pair p's windows;
        # proj half A (heads 0-3, complete after pair 1) fills pair 3.
        # HAM warm-up: keep the PE busy while the first input chunks are
        # still in flight so the clock gate opens before the real work.
        wps = scp.tile([128, 1024], F32, tag='sc', name='warmps')
        for i in range(12):
            nc.tensor.matmul(wps[:, 0:128], identb[:], identb[:],
                             start=True, stop=True)
        g0 = gen_qk(0, wqt, QT2[0], _copier(nc.scalar),
                    ec_order=[4, 5, 6, 7, 0, 1, 2, 3])
        for _ in g0:
            for i in range(6):
                nc.tensor.matmul(wps[:, 0:128], identb[:], identb[:],
                                 start=True, stop=True)
        for st in range(4):
            drain(gen_v(st))
        drain(gen_qk(0, wkt, KT2[0], _copier(nc.scalar)))

        # ---- fillers for the attention phase ----------------------------
        per_pair_fillers = {
            0: [gen_v(4), gen_v(5), gen_v(6), gen_v(7),
                gen_qk(1, wqt, QT2[1], _copier(nc.scalar), filler=True),
                gen_qk(1, wkt, KT2[1], _copier(nc.scalar), filler=True)],
            1: [gen_qk(2, wqt, QT2[2], _copier(nc.vector), filler=True),
                gen_qk(2, wkt, KT2[2], _copier(nc.vector), filler=True)],
            2: [gen_qk(3, wqt, QT2[3], _copier(nc.vector), filler=True),
                gen_qk(3, wkt, KT2[3], _copier(nc.vector), filler=True)],
            3: [gen_proj_half(0), gen_projB_p1()],
        }

        # ---- attention ---------------------------------------------------
        CA3 = CA[:].rearrange('p (st c) -> p st c', st=8)
        CB3 = CB[:].rearrange('p (st c) -> p st c', st=8)
        tri_bc = tri[:, None, :].broadcast_to((128, 2, 128))

        def emit_scores(p, qc, kb):
            """Row-tiled pair of score matmuls + exp + causal mask.
            Returns the pexp tile."""
            j = kb - 4 * qc
            off = 128 * j if j >= 0 else 0
            stp = scp.tile([128, 1024], F32, tag='sc',
                           name=f'stp_{p}_{qc}_{kb}')
            for hh in range(2):
                nc.tensor.matmul(
                    stp[:, hh * 512 + off:(hh + 1) * 512],
                    KT2[p][64 * hh:64 * hh + 64, kb * 128:(kb + 1) * 128],
                    QT2[p][64 * hh:64 * hh + 64,
                           qc * 512 + off:(qc + 1) * 512],
                    start=True, stop=True)
            pexp = next_slot('pe', pexp_slots)
            stp3 = stp[:].rearrange('p (two c) -> p two c', two=2)
            pexp3 = pexp[:].rearrange('p (two c) -> p two c', two=2)
            nc.scalar.activation(pexp3[:, :, off:512], stp3[:, :, off:512],
                                 AF.Exp, scale=0.125)
            if j >= 0:
                nc.vector.tensor_mul(pexp3[:, :, off:off + 128],
                                     pexp3[:, :, off:off + 128], tri_bc)
            return pexp, off

        pending_tp = []

        def emit_pending_tp():
            for fn in pending_tp:
                fn()
            pending_tp.clear()

        def defer_transposes(p, qc, otsbs):
            def go():
                for hh in range(2):
                    h = 2 * p + hh
                    tp = avp.tile([128, 512], F32, tag='av',
                                  name=f'tp_{p}_{qc}_{hh}')
                    for qb in range(4):
                        nc.tensor.transpose(
                            tp[:, qb * 65:qb * 65 + 65],
                            otsbs[hh][:D + 1, qb * 128:(qb + 1) * 128],
                            identf[:D + 1, :D + 1])
                    tp3 = tp[:, 0:260].rearrange('p (qb c) -> p qb c', qb=4)
                    rl = next_slot('rl', rl_slots)
                    nc.vector.reciprocal(rl[:], tp3[:, :, D])
                    Cd3, hl = (CA3, h) if h < 6 else (CB3, h - 6)
                    nc.vector.tensor_mul(
                        Cd3[:, qc * 4:(qc + 1) * 4, hl * D:(hl + 1) * D],
                        tp3[:, :, 0:D],
                        rl[:, :, None].broadcast_to((128, 4, D)))
            pending_tp.append(go)

        for p in range(4):
            if p >= 1:
                while fillers:   # QK(p) leftovers must land before pair p
                    fill(1)
            for g in per_pair_fillers.get(p, []):
                fillers.append(g)
            for qc in range(2):
                if p == 0 and qc == 1:
                    # V4..7 must be fully emitted before qc1 reads them
                    while len(fillers) > 2:
                        fill(1)
                kbs = list(range(4 * qc + 4))
                n = len(kbs)
                pend = {}
                pend[0] = emit_scores(p, qc, 0)
                if n > 1:
                    pend[1] = emit_scores(p, qc, 1)
                # previous window's transposes land here, covering the
                # exp latency of this window's first blocks
                emit_pending_tp()
                ots = [avp.tile([128, 512], F32, tag='av',
                                name=f'ot_{p}_{qc}_{hh}') for hh in range(2)]
                for t in range(n):
                    fill(1)
                    pexp, off = pend.pop(t)
                    for hh in range(2):
                        nc.tensor.matmul(
                            ots[hh][:D + 1, off:512],
                            Vall[t][:, (2 * p + hh) * (D + 1):
                                    (2 * p + hh + 1) * (D + 1)],
                            pexp[:, hh * 512 + off:(hh + 1) * 512],
                            start=(t == 0), stop=(t == n - 1))
                    if t + 2 < n:
                        pend[t + 2] = emit_scores(p, qc, t + 2)
                # drain accumulators now (releases av tiles); defer the
                # PE transposes into the next window
                otsbs = []
                for hh in range(2):
                    otsb = next_slot('ob', otsb_slots)
                    nc.vector.tensor_copy(otsb[:D + 1, :],
                                          ots[hh][:D + 1, :])
                    otsbs.append(otsb)
                defer_transposes(p, qc, otsbs)

        while fillers:       # leftover phase-1 units (PE-dense)
            fill(1)
        emit_pending_tp()
        drain(gen_projB_p2())

        if dbg is not None:
            nc.sync.dma_start(dbg['dC'][:, 0:8 * 384], CA[:])
            nc.sync.dma_start(dbg['dC'][:, 8 * 384:], CB[:])
            for p4 in range(4):
                nc.sync.dma_start(dbg['dQ'][p4 * 128:(p4 + 1) * 128, :],
                                  QT2[p4][:])
                nc.sync.dma_start(dbg['dK'][p4 * 128:(p4 + 1) * 128, :],
                                  KT2[p4][:])
            for st in range(8):
                nc.sync.dma_start(dbg['dV'][st * 128:(st + 1) * 128, :],
                                  Vall[st][:])




_NC_CACHE = None


def _get_nc():
    global _NC_CACHE
    if _NC_CACHE is None:
        _NC_CACHE = build_nc()
    return _NC_CACHE


def make_in_maps(x, Wq, Wk, Wv, W_O):
    import ml_dtypes
    bf = ml_dtypes.bfloat16
    x = np.asarray(x, np.float32)
    xT_by_b = [np.ascontiguousarray(x[b].T.astype(bf)) for b in range(4)]
    W_O = np.ascontiguousarray(np.asarray(W_O, np.float32).T.astype(bf))
    in_maps = []
    for c in range(8):
        b, g = c // 2, c % 2
        hsl = slice(HC * g, HC * g + HC)
        in_maps.append({
            'xb': xT_by_b[b],
            'wq': np.ascontiguousarray(
                np.asarray(Wq, np.float32)[hsl].transpose(1, 0, 2)
                .reshape(E, HC * D).astype(bf)),
            'wk': np.ascontiguousarray(
                np.asarray(Wk, np.float32)[hsl].transpose(1, 0, 2)
                .reshape(E, HC * D).astype(bf)),
            'wv': np.ascontiguousarray(
                np.asarray(Wv, np.float32)[hsl].transpose(1, 0, 2)
                .reshape(E, HC * D).astype(bf)),
            'wo': W_O,
        })
    return in_maps


def kernel(x, Wq, Wk, Wv, W_O):
    from concourse.bass_utils import run_bass_kernel_spmd
    nc = _get_nc()
    in_maps = make_in_maps(x, Wq, Wk, Wv, W_O)
    res = run_bass_kernel_spmd(nc, in_maps, list(range(8)))
    full = np.empty((4, E, E), np.float32)
    for c in range(8):
        b, g = c // 2, c % 2
        full[b, :, NO * g:NO * g + NO] = res.results[c]['out']
    return full


# revision 34
# speedup vs baseline: 1.0486x; 1.0118x over previous
"""Trainium2 Bass kernel v2 for nn_MultiHeadAttention_8667244003725.

B=4, S=1024, E=1024, H=16, D=64.  Same sharding as baseline:
8 cores = 4 batches x 2 head-groups; core c -> out[b, :, 512g:512g+512],
b=c//2, g=c%2.  No collectives.

v2 changes vs baseline (173us):
- scores row-tiled: both heads of a pair run CONCURRENTLY on the PE
  (K=64 contractions at tile rows 0-63 / 64-127) into one [128,1024] slab.
- one exp per (pair,qc,kb) covering both heads via strided AP; one DVE
  tri-mask per diagonal pair (broadcast AP).
- QKV jobs for later pairs emitted as filler units inside the attention
  chain so the PE never idles on exp -> HAM stays at 2.4 GHz.
- transposes: f32 [65,128] blocks into a shared psum ring; normalize via
  one reciprocal [128,4] + one broadcast tensor_mul per (head, qc).
- weight DMAs off the scalar queue; V/Q/K psum drains on gpsimd/scalar.
"""

import sys

if '/opt/trn_rl_repo' not in sys.path:
    sys.path.insert(0, '/opt/trn_rl_repo')

import numpy as np

import concourse.bass as bass
import concourse.mybir as mybir
import concourse.tile as tile
from concourse.masks import make_identity

F32 = mybir.dt.float32
BF = mybir.dt.bfloat16
AF = mybir.ActivationFunctionType
MUL = mybir.AluOpType.mult

S = 1024
E = 1024
D = 64
HC = 8            # heads per core
NO = 512          # output columns per core


def _split_sync_waits(nc, limit=1):
    """Walrus here rejects >1 sem-wait per instruction; hoist extras onto
    same-engine no-ops."""
    n = 0
    for f in nc.m.functions:
        for bb in f.blocks:
            out = []
            for ins in bb.instructions:
                si = ins.sync_info
                waits = list(si.on_wait) if si is not None else []
                if len(waits) > limit:
                    excess, keep = waits[:-limit], waits[-limit:]
                    for i in range(0, len(excess), limit):
                        grp = excess[i:i + limit]
                        n += 1
                        out.append(mybir.InstNoOp(
                            name=f'I-synsplit-{n}', ins=[], outs=[],
                            engine=ins.engine,
                            sync_info=mybir.SyncInfo(on_wait=list(grp),
                                                     on_update=[])))
                    si.on_wait = keep
                out.append(ins)
            bb.instructions = out
    return n


def build_nc(split_waits=True, debug=False):
    nc = bass.Bass()
    xb = nc.dram_tensor('xb', [E, S], BF, kind='ExternalInput')   # x[b]^T
    wq = nc.dram_tensor('wq', [E, HC * D], BF, kind='ExternalInput')
    wk = nc.dram_tensor('wk', [E, HC * D], BF, kind='ExternalInput')
    wv = nc.dram_tensor('wv', [E, HC * D], BF, kind='ExternalInput')
    wo = nc.dram_tensor('wo', [E, E], BF, kind='ExternalInput')   # W_O^T
    out = nc.dram_tensor('out', [E, NO], F32, kind='ExternalOutput')
    dbg = None
    if debug:
        dbg = {
            'dC': nc.dram_tensor('dC', [128, 8 * NO], BF,
                                 kind='ExternalOutput'),
            'dQ': nc.dram_tensor('dQ', [4 * 128, S], BF,
                                 kind='ExternalOutput'),
            'dK': nc.dram_tensor('dK', [4 * 128, S], BF,
                                 kind='ExternalOutput'),
            'dV': nc.dram_tensor('dV', [8 * 128, HC * (D + 1)], BF,
                                 kind='ExternalOutput'),
        }

    with tile.TileContext(nc) as tc:
        _emit(nc, tc, xb, wq, wk, wv, wo, out, dbg=dbg)
    if split_waits:
        _split_sync_waits(nc)
    return nc


def _copier(eng):
    """Uniform copy callable: scalar uses activation-Copy, DVE tensor_copy.
    (GPSIMD cannot read PSUM on trn2.)"""
    if hasattr(eng, 'tensor_copy'):
        return eng.tensor_copy
    return eng.copy


def _emit(nc, tc, xb, wq, wk, wv, wo, out, dbg=None):
    with (
        tc.tile_pool(name='const', bufs=1) as constp,
        tc.tile_pool(name='big', bufs=1) as bigp,      # xT, weights, WOT, C
        tc.tile_pool(name='qk', bufs=1) as qkp,
        tc.tile_pool(name='vall', bufs=1) as vallp,
        tc.tile_pool(name='sc', bufs=2, space='PSUM') as scp,   # 4 banks
        tc.tile_pool(name='av', bufs=3, space='PSUM') as avp,   # 3 banks
        tc.tile_pool(name='fl', bufs=1, space='PSUM') as flp,   # 1 bank
    ):
        # ---- constants --------------------------------------------------
        identf = constp.tile([128, 128], F32, tag='identf')
        make_identity(nc, identf[:])
        identb = constp.tile([128, 128], BF, tag='identb')
        make_identity(nc, identb[:])
        ones8 = constp.tile([128, 8], BF, tag='ones8')
        nc.gpsimd.memset(ones8[:], 1.0)
        # tri[k, q] = 1 where q >= k else 0 (multiplicative causal mask)
        tri = constp.tile([128, 128], BF, tag='tri')
        nc.gpsimd.memset(tri[:], 1.0)
        nc.gpsimd.affine_select(
            out=tri[:], in_=tri[:], compare_op=mybir.AluOpType.is_ge,
            fill=0.0, base=0, channel_multiplier=-1, pattern=[[1, 128]])
        # ---- input DMAs -------------------------------------------------
        # Per-chunk tiles so consumers depend on exactly the chunk they
        # read (whole-tile deps made the first matmul wait ~16us).
        # Queues:  sync x4..7 + wk0..7;  scalar wq4..7,0..3;
        #          gpsimd x0..3 + wv + wo.
        xTc = [bigp.tile([128, S], BF, tag=f'x{ec}', name=f'xT{ec}')
               for ec in range(8)]
        wqc = [bigp.tile([128, 512], BF, tag=f'wq{ec}', name=f'wq{ec}')
               for ec in range(8)]
        wkc = [bigp.tile([128, 512], BF, tag=f'wk{ec}', name=f'wk{ec}')
               for ec in range(8)]
        wvc = [bigp.tile([128, 512], BF, tag=f'wv{ec}', name=f'wv{ec}')
               for ec in range(8)]
        for ec in (4, 5, 6, 7, 0, 1, 2, 3):
            nc.sync.dma_start(xTc[ec][:], xb[ec * 128:(ec + 1) * 128, :])
            nc.scalar.dma_start(wqc[ec][:], wq[ec * 128:(ec + 1) * 128, :])
            nc.gpsimd.dma_start(wvc[ec][:], wv[ec * 128:(ec + 1) * 128, :])
        for ec in range(8):
            nc.sync.dma_start(wkc[ec][:], wk[ec * 128:(ec + 1) * 128, :])
        # warm the ACT exp table after the scalar-queue DMAs are issued
        warm = constp.tile([1, 2], F32, tag='warm')
        nc.scalar.activation(warm[:], ones8[0:1, 0:2], AF.Exp, scale=0.125)
        xT = [t[:] for t in xTc]
        wqt = [t[:] for t in wqc]
        wkt = [t[:] for t in wkc]
        wvt = [t[:] for t in wvc]

        # ---- persistent SBUF --------------------------------------------
        QT2 = [qkp.tile([128, S], BF, tag=f'q{p}', name=f'QT2_{p}')
               for p in range(4)]
        KT2 = [qkp.tile([128, S], BF, tag=f'k{p}', name=f'KT2_{p}')
               for p in range(4)]
        Vall = [vallp.tile([128, HC * (D + 1)], BF, tag=f'v{st}',
                           name=f'Vall{st}') for st in range(8)]
        CA = bigp.tile([128, 8 * 384], BF, tag='CA', name='CA')  # heads 0-5
        CB = bigp.tile([128, 8 * 128], BF, tag='CB', name='CB')  # heads 6-7
        WOTall = bigp.tile([128, 8 * E], BF, tag='WOT', name='WOTall')
        PB = bigp.tile([128, 8 * 128], F32, tag='PB', name='PBpart')
        # persistent cycled slots (WAR deps give the same ordering a pool
        # ring would, without per-allocation release-semaphore traffic)
        pexp_slots = [bigp.tile([128, 1024], BF, tag=f'pe{i}',
                                name=f'pexp{i}') for i in range(4)]
        otsb_slots = [bigp.tile([128, 512], F32, tag=f'ob{i}',
                                name=f'otsb{i}') for i in range(4)]
        rl_slots = [bigp.tile([128, 4], F32, tag=f'rl{i}', name=f'rl{i}')
                    for i in range(2)]
        ys_slots = [bigp.tile([128, 384], F32, tag=f'ys{i}', name=f'ys{i}')
                    for i in range(4)]
        slot_ctr = {'pe': 0, 'ob': 0, 'rl': 0, 'ys': 0}

        def next_slot(kind, slots):
            i = slot_ctr[kind] % len(slots)
            slot_ctr[kind] += 1
            return slots[i]
        WOT = [WOTall[:, i * E:(i + 1) * E] for i in range(8)]
        for i in range(8):
            nc.gpsimd.dma_start(WOTall[:, i * E:(i + 1) * E],
                                wo[i * 128:(i + 1) * 128, :])

        # ---- job generators (each yield = ~0.4-0.9us of PE work) --------
        def gen_qk(p, wt, dst, copy_eng, ec_order=None, filler=False):
            ecs = ec_order or list(range(8))
            if not filler:
                ps = scp.tile([128, 1024], F32, tag='sc',
                              name=f'qk_{p}_{wt is wkt}')
                for i, ec in enumerate(ecs):
                    for s2 in range(2):
                        nc.tensor.matmul(
                            ps[:, s2 * 512:(s2 + 1) * 512],
                            wt[ec][:, p * 128:(p + 1) * 128],
                            xT[ec][:, s2 * 512:(s2 + 1) * 512],
                            start=(i == 0), stop=(i == 7))
                    if i % 2 == 1 and i < 7:
                        yield
                copy_eng(dst[:], ps[:])
                yield
                return
            # filler mode: isolated single-bank pool, two half chains
            for s2 in range(2):
                ps = flp.tile([128, 512], F32, tag='fl',
                              name=f'qkf_{p}_{wt is wkt}_{s2}')
                for i, ec in enumerate(ecs):
                    nc.tensor.matmul(
                        ps[:], wt[ec][:, p * 128:(p + 1) * 128],
                        xT[ec][:, s2 * 512:(s2 + 1) * 512],
                        start=(i == 0), stop=(i == 7))
                    if i % 2 == 1 and i < 7:
                        yield
                copy_eng(dst[:, s2 * 512:(s2 + 1) * 512], ps[:])
                yield

        def gen_v(st):
            ps = scp.tile([128, 1024], F32, tag='sc', name=f'v_{st}')
            for i in range(8):
                nc.tensor.matmul(ps[:, 0:512],
                                 xT[i][:, st * 128:(st + 1) * 128],
                                 wvt[i], start=(i == 0), stop=(i == 7))
                if i % 2 == 1 and i < 7:
                    yield
            v3 = Vall[st][:].rearrange('p (h d) -> p h d', h=HC)
            cp = _copier(nc.vector)
            cp(v3[:, :, 0:D],
               ps[:, 0:512].rearrange('p (h d) -> p h d', h=HC))
            cp(v3[:, :, D:D + 1],
               ones8[:].rearrange('p (h o) -> p h o', o=1))
            yield

        def drain(g):
            for _ in g:
                pass

        from collections import deque
        fillers = deque()

        def fill(n=1):
            k = 0
            while k < n and fillers:
                try:
                    next(fillers[0])
                    k += 1
                except StopIteration:
                    fillers.popleft()

        def gen_projB_p1():
            # proj for heads 6-7, contraction s-blocks 0..3 only (ready
            # after pair-3 qc0); partial parked in SBUF
            for it in range(8):
                ps = flp.tile([128, 512], F32, tag='fl', name=f'pb1_{it}')
                for i in range(4):
                    nc.tensor.matmul(ps[:, 0:128],
                                     WOT[i][:, it * 128:(it + 1) * 128],
                                     CB[:, i * 128:(i + 1) * 128],
                                     start=(i == 0), stop=(i == 3))
                nc.vector.tensor_copy(PB[:, it * 128:(it + 1) * 128],
                                      ps[:, 0:128])
                yield

        def gen_projB_p2():
            for it in range(8):
                ps = scp.tile([128, 1024], F32, tag='sc', name=f'pb2_{it}')
                for i in range(4, 8):
                    nc.tensor.matmul(ps[:, 0:128],
                                     WOT[i][:, it * 128:(it + 1) * 128],
                                     CB[:, i * 128:(i + 1) * 128],
                                     start=(i == 4), stop=(i == 7))
                ys = next_slot('ys', ys_slots)
                nc.vector.tensor_add(ys[:, 0:128], ps[:, 0:128],
                                     PB[:, it * 128:(it + 1) * 128])
                dq = nc.sync if it % 2 == 0 else nc.scalar
                dq.dma_start(out[it * 128:(it + 1) * 128, 384:512],
                             ys[:, 0:128])
                yield

        def gen_proj_half(half):
            """Output projection: half 0 = 384 cols (heads 0-5, complete
            after pair 2 -> filler for pair 3); half 1 = 128 cols (heads
            6-7) at the end.  One whole it-chunk per unit."""
            Csrc, j0, w = (CA, 0, 384) if half == 0 else (CB, 384, 128)
            for it in range(8):
                if half == 0:
                    ps = flp.tile([128, 512], F32, tag='fl',
                                  name=f'proj{half}_{it}')
                else:
                    ps = scp.tile([128, 1024], F32, tag='sc',
                                  name=f'proj{half}_{it}')
                for i in range(8):
                    nc.tensor.matmul(ps[:, 0:w],
                                     WOT[i][:, it * 128:(it + 1) * 128],
                                     Csrc[:, i * w:(i + 1) * w],
                                     start=(i == 0), stop=(i == 7))
                ys = next_slot('ys', ys_slots)
                cp = _copier(nc.vector if (half == 0 or it % 2 == 0)
                             else nc.scalar)
                cp(ys[:, 0:w], ps[:, 0:w])
                dq = nc.sync if it % 2 == 0 else nc.scalar
                dq.dma_start(out[it * 128:(it + 1) * 128, j0:j0 + w],
                             ys[:, 0:w])
                yield

        # ---- prologue: Q0 V0..7 K0 (PE-dense, warms HAM) ----------------
        # Rule (empirical, HW): a filler's output must be fully emitted
        # before the first instruction of the window that consumes it.
        # So V is all in the prologue; QK(p+1) fills pair p's windows;
        # proj half A (heads 0-3, complete after pair 1) fills pair 3.
        # HAM warm-up: keep the PE busy while the first input chunks are
        # still in flight so the clock gate opens before the real work.
        wps = scp.tile([128, 1024], F32, tag='sc', name='warmps')
        for i in range(12):
            nc.tensor.matmul(wps[:, 0:128], identb[:], identb[:],
                             start=True, stop=True)
        g0 = gen_qk(0, wqt, QT2[0], _copier(nc.scalar),
                    ec_order=[4, 5, 6, 7, 0, 1, 2, 3])
        for _ in g0:
            for i in range(6):
                nc.tensor.matmul(wps[:, 0:128], identb[:], identb[:],
                                 start=True, stop=True)
        for st in range(4):
            drain(gen_v(st))
        drain(gen_qk(0, wkt, KT2[0], _copier(nc.scalar)))

        # ---- fillers for the attention phase ----------------------------
        per_pair_fillers = {
            0: [gen_v(4), gen_v(5), gen_v(6), gen_v(7),
                gen_qk(1, wqt, QT2[1], _copier(nc.scalar), filler=True),
                gen_qk(1, wkt, KT2[1], _copier(nc.scalar), filler=True)],
            1: [gen_qk(2, wqt, QT2[2], _copier(nc.vector), filler=True),
                gen_qk(2, wkt, KT2[2], _copier(nc.vector), filler=True)],
            2: [gen_qk(3, wqt, QT2[3], _copier(nc.vector), filler=True),
                gen_qk(3, wkt, KT2[3], _copier(nc.vector), filler=True)],
            3: [gen_proj_half(0), gen_projB_p1()],
        }

        # ---- attention ---------------------------------------------------
        CA3 = CA[:].rearrange('p (st c) -> p st c', st=8)
        CB3 = CB[:].rearrange('p (st c) -> p st c', st=8)
        tri_bc = tri[:, None, :].broadcast_to((128, 2, 128))

        def emit_scores(p, qc, kb):
            """Row-tiled pair of score matmuls + exp + causal mask.
            Returns the pexp tile."""
            j = kb - 4 * qc
            off = 128 * j if j >= 0 else 0
            stp = scp.tile([128, 1024], F32, tag='sc',
                           name=f'stp_{p}_{qc}_{kb}')
            for hh in range(2):
                nc.tensor.matmul(
                    stp[:, hh * 512 + off:(hh + 1) * 512],
                    KT2[p][64 * hh:64 * hh + 64, kb * 128:(kb + 1) * 128],
                    QT2[p][64 * hh:64 * hh + 64,
                           qc * 512 + off:(qc + 1) * 512],
                    start=True, stop=True)
            pexp = next_slot('pe', pexp_slots)
            stp3 = stp[:].rearrange('p (two c) -> p two c', two=2)
            pexp3 = pexp[:].rearrange('p (two c) -> p two c', two=2)
            nc.scalar.activation(pexp3[:, :, off:512], stp3[:, :, off:512],
                                 AF.Exp, scale=0.125)
            if j >= 0:
                nc.vector.tensor_mul(pexp3[:, :, off:off + 128],
                                     pexp3[:, :, off:off + 128], tri_bc)
            return pexp, off

        pending_tp = []
        prefetched = {}

        def emit_pending_tp():
            for fn in pending_tp:
                fn()
            pending_tp.clear()

        def defer_transposes(p, qc, otsbs):
            def go():
                for hh in range(2):
                    h = 2 * p + hh
                    tp = avp.tile([128, 512], F32, tag='av',
                                  name=f'tp_{p}_{qc}_{hh}')
                    for qb in range(4):
                        nc.tensor.transpose(
                            tp[:, qb * 65:qb * 65 + 65],
                            otsbs[hh][:D + 1, qb * 128:(qb + 1) * 128],
                            identf[:D + 1, :D + 1])
                    tp3 = tp[:, 0:260].rearrange('p (qb c) -> p qb c', qb=4)
                    rl = next_slot('rl', rl_slots)
                    nc.vector.reciprocal(rl[:], tp3[:, :, D])
                    Cd3, hl = (CA3, h) if h < 6 else (CB3, h - 6)
                    nc.vector.tensor_mul(
                        Cd3[:, qc * 4:(qc + 1) * 4, hl * D:(hl + 1) * D],
                        tp3[:, :, 0:D],
                        rl[:, :, None].broadcast_to((128, 4, D)))
            pending_tp.append(go)

        for p in range(4):
            if p >= 1:
                while fillers:   # QK(p) leftovers must land before pair p
                    fill(1)
            for g in per_pair_fillers.get(p, []):
                fillers.append(g)
            for qc in range(2):
                if p == 0 and qc == 1:
                    # V4..7 must be fully emitted before qc1 reads them
                    while len(fillers) > 2:
                        fill(1)
                kbs = list(range(4 * qc + 4))
                n = len(kbs)
                pend = prefetched.pop((p, qc), {})
                if 0 not in pend:
                    pend[0] = emit_scores(p, qc, 0)
                if n > 1 and 1 not in pend:
                    pend[1] = emit_scores(p, qc, 1)
                # previous window's transposes land here, covering the
                # exp latency of this window's first blocks
                emit_pending_tp()
                ots = [avp.tile([128, 512], F32, tag='av',
                                name=f'ot_{p}_{qc}_{hh}') for hh in range(2)]
                for t in range(n):
                    fill(2 if qc == 1 else 1)
                    pexp, off = pend.pop(t)
                    if t == n - 1 and qc == 0:
                        # prefetch qc1's first score pairs ahead of the
                        # last AV: its exp starts while this window drains
                        # (same-pair only -- QT/KT are long since final)
                        prefetched[(p, 1)] = {
                            0: emit_scores(p, 1, 0),
                            1: emit_scores(p, 1, 1),
                        }
                    for hh in range(2):
                        nc.tensor.matmul(
                            ots[hh][:D + 1, off:512],
                            Vall[t][:, (2 * p + hh) * (D + 1):
                                    (2 * p + hh + 1) * (D + 1)],
                            pexp[:, hh * 512 + off:(hh + 1) * 512],
                            start=(t == 0), stop=(t == n - 1))
                    if t + 2 < n:
                        pend[t + 2] = emit_scores(p, qc, t + 2)
                # drain accumulators now (releases av tiles); defer the
                # PE transposes into the next window
                otsbs = []
                for hh in range(2):
                    otsb = next_slot('ob', otsb_slots)
                    nc.vector.tensor_copy(otsb[:D + 1, :],
                                          ots[hh][:D + 1, :])
                    otsbs.append(otsb)
                defer_transposes(p, qc, otsbs)

        while fillers:       # leftover phase-1 units (PE-dense)
            fill(1)
        emit_pending_tp()
        drain(gen_projB_p2())

        if dbg is not None:
            nc.sync.dma_start(dbg['dC'][:, 0:8 * 384], CA[:])
            nc.sync.dma_start(dbg['dC'][:, 8 * 384:], CB[:])
            for p4 in range(4):
                nc.sync.dma_start(dbg['dQ'][p4 * 128:(p4 + 1) * 128, :],
                                  QT2[p4][:])
                nc.sync.dma_start(dbg['dK'][p4 * 128:(p4 + 1) * 128, :],
                                  KT2[p4][:])
            for st in range(8):
                nc.sync.dma_start(dbg['dV'][st * 128:(st + 1) * 128, :],
                                  Vall[st][:])




_NC_CACHE = None


def _get_nc():
    global _NC_CACHE
    if _NC_CACHE is None:
        _NC_CACHE = build_nc()
    return _NC_CACHE


def make_in_maps(x, Wq, Wk, Wv, W_O):
    import ml_dtypes
    bf = ml_dtypes.bfloat16
    x = np.asarray(x, np.float32)
    xT_by_b = [np.ascontiguousarray(x[b].T.astype(bf)) for b in range(4)]
    W_O = np.ascontiguousarray(np.asarray(W_O, np.float32).T.astype(bf))
    in_maps = []
    for c in range(8):
        b, g = c // 2, c % 2
        hsl = slice(HC * g, HC * g + HC)
        in_maps.append({
            'xb': xT_by_b[b],
            'wq': np.ascontiguousarray(
                np.asarray(Wq, np.float32)[hsl].transpose(1, 0, 2)
                .reshape(E, HC * D).astype(bf)),
            'wk': np.ascontiguousarray(
                np.asarray(Wk, np.float32)[hsl].transpose(1, 0, 2)
                .reshape(E, HC * D).astype(bf)),
            'wv': np.ascontiguousarray(
                np.asarray(Wv, np.float32)[hsl].transpose(1, 0, 2)
                .reshape(E, HC * D).astype(bf)),
            'wo': W_O,
        })
    return in_maps


def kernel(x, Wq, Wk, Wv, W_O):
    from concourse.bass_utils import run_bass_kernel_spmd
    nc = _get_nc()
    in_maps = make_in_maps(x, Wq, Wk, Wv, W_O)
    res = run_bass_kernel_spmd(nc, in_maps, list(range(8)))
    full = np.empty((4, E, E), np.float32)
    for c in range(8):
        b, g = c // 2, c % 2
        full[b, :, NO * g:NO * g + NO] = res.results[c]['out']
    return full
